# revision 3
# baseline (speedup 1.0000x reference)
"""Trainium2 Bass kernel for ClassCentersEMA (vq_codebook) — v3.

Reference semantics (B=16384, D=1024, C=512):
    feats_n   = feats / max(||feats||_row, eps)
    counts    = targets.sum(0)                       # [C]
    class_sums= targets^T @ feats_n                  # [C, D]
    mask      = counts > 0
    means     = class_sums / max(counts, 1)          # rows 0 where !mask
    new       = !initialized & mask
    base      = where(new, means, centers)
    blended   = 0.9*base + 0.1*means
    upd       = where(mask, blended, centers)
    out       = where(mask, upd / max(||upd||, eps), upd)

v3 vs the v2 DRAM-AllToAll design (59us):
  - Host pre-casts feats and targets to bf16: the HBM input stream drops
    from 12.3 MiB to 6.25 MiB per core; the PE bf16 matmul (~28us)
    becomes the critical path instead of DMA.
  - The cross-core reduction of the [C, D] partials goes SBUF->SBUF via
    remote_dma_broadcast (SWDGE) instead of bouncing 8.3 MiB per core
    through HBM + an ncfw AllToAll. targets columns are permuted on the
    HOST per core (position block d holds the classes of core r^d), so
    every core runs the identical program and relative XOR destinations
    line up with class ownership: slab c (psum pair c) rows 0:64 belong
    to relative dest 2c, rows 64:128 to dest 2c+1; core r^d receives my
    slab at recv[d] and uses the half selected by d's parity.
  - Counts ride as one extra bf16 column (exact: per-core partial counts
    are small integers << 256) on each 128x1028 slab.
  - The exchange, counts matmuls, psum drains and the 8-way reduction
    live in a tc.tile_critical() section with manual semaphores (Tile's
    build-time scheduling sim cannot model cross-core semaphore
    increments; inside a critical section engines run FIFO and the sim
    does not trace the waits).
  - A tiny AllGather at NEFF start is a cross-core rendezvous: it
    guarantees every core has started (and had its semaphores reset)
    before any remote increment can land. gpsimd-only, overlapped with
    the input stream, once per dispatch.
"""

import numpy as np
import ml_dtypes

import concourse.bass as bass
import concourse.mybir as mybir
import concourse.tile as tile
from concourse import bacc
from concourse.bass_utils import run_bass_kernel_spmd

F32 = mybir.dt.float32
BF16 = mybir.dt.bfloat16
AF = mybir.ActivationFunctionType
ALU = mybir.AluOpType

NCORES = 8
B, D, C = 16384, 1024, 512
BL = B // NCORES          # 2048 rows per core
KT = BL // 128            # 16 k-tiles of 128
CL = C // NCORES          # 64 classes per core
MOM = 0.9
EPS = 1e-12
SW = 1028                 # slab width: 1024 D + 1 counts + 3 pad


def build_nc(niters=1, use_coll=True):
    """niters>1 unrolls the kernel body N times in one NEFF — used only
    for timing (slope over N isolates device exec time)."""
    nc = bacc.Bacc("TRN2", target_bir_lowering=False, debug=False,
                   num_devices=NCORES)

    feats = nc.dram_tensor("feats", [BL, D], BF16, kind="ExternalInput")
    targets = nc.dram_tensor("targets", [BL, C], BF16, kind="ExternalInput")
    centers = nc.dram_tensor("centers", [CL, D], F32, kind="ExternalInput")
    inited = nc.dram_tensor("inited", [CL, 1], F32, kind="ExternalInput")
    out = nc.dram_tensor("out", [CL, D], F32, kind="ExternalOutput")

    rg = [list(range(NCORES))]

    sems = dict(
        arr=[nc.alloc_semaphore(f"arr{d}") for d in range(NCORES)],
        sent=nc.alloc_semaphore("sent"),
        sent2=nc.alloc_semaphore("sent2"),
        credit=nc.alloc_semaphore("credit"),
        prep=nc.alloc_semaphore("prep"),
        evn=nc.alloc_semaphore("evn"),
    )

    with tile.TileContext(nc) as tc:
        with (
            tc.tile_pool(name="dram", bufs=1, space="DRAM") as dram,
            tc.tile_pool(name="ftp", bufs=3) as ftp,
            tc.tile_pool(name="tgp", bufs=1) as tgp,
            tc.tile_pool(name="tsp", bufs=3) as tsp,
            tc.tile_pool(name="sq", bufs=2) as sqp,
            tc.tile_pool(name="small", bufs=6) as small,
            tc.tile_pool(name="single", bufs=1) as single,
            tc.tile_pool(name="slab", bufs=2) as slabp,
            tc.tile_pool(name="recv", bufs=1) as recvp,
            tc.tile_pool(name="red", bufs=1) as redp,
            tc.tile_pool(name="psum", bufs=8, space="PSUM") as ppool,
            tc.tile_pool(name="epi", bufs=1) as epi,
        ):
            io = dict(feats=feats, targets=targets, centers=centers,
                      inited=inited, out=out)

            ones = single.tile([128, 1], BF16, name="ones")
            nc.vector.memset(ones[:], 1.0)
            eps2 = single.tile([128, 1], F32, name="eps2")
            nc.vector.memset(eps2[:], EPS * EPS)
            consts = dict(ones=ones, eps2=eps2)

            # entry rendezvous (see module docstring)
            bar_in = dram.tile([1, 64], F32, name="bar_in")
            bar_out = dram.tile([NCORES, 64], F32, name="bar_out")
            nc.gpsimd.collective_compute(
                "AllGather", ALU.bypass, replica_groups=rg,
                ins=[bar_in[:].opt()], outs=[bar_out[:].opt()])

            # persistent recv tiles — remote cores write here; addresses
            # are identical on every core (identical program). Two parity
            # sets: iteration it uses set it%2, so sends need peers'
            # consumption of it-2 (not it-1) — the credit roundtrip
            # overlaps a full iteration.
            recvs = [[None] + [recvp.tile([128, SW], BF16,
                                          name=f"recv{par}_{d}")
                               for d in range(1, NCORES)]
                     for par in range(2)]

            pools = dict(ftp=ftp, tgp=tgp, tsp=tsp, sqp=sqp, small=small,
                         slabp=slabp, redp=redp, ppool=ppool, epi=epi)
            for i in range(niters):
                _emit_iteration(nc, tc, i, io, consts, pools, recvs, sems)

    nc.compile()
    return nc


def _emit_iteration(nc, tc, it, io, consts, pools, recvs, sems):
    feats, targets = io["feats"], io["targets"]
    centers, inited, out = io["centers"], io["inited"], io["out"]
    ones, eps2 = consts["ones"], consts["eps2"]
    ftp, tgp, tsp = pools["ftp"], pools["tgp"], pools["tsp"]
    sqp, small = pools["sqp"], pools["small"]
    slabp, redp, ppool, epi = (pools["slabp"], pools["redp"], pools["ppool"],
                               pools["epi"])
    arr = sems["arr"]

    # epilogue inputs that depend on nothing — issue DMAs up front
    ctr = epi.tile([CL, D], F32, tag="ctr")
    nc.scalar.dma_start(ctr[:], centers[:])
    ini = epi.tile([CL, 1], F32, tag="ini")
    nc.scalar.dma_start(ini[:], inited[:])

    # 8 PSUM banks: (position-block pair, D-half) over all 16 k-tiles
    ps = [ppool.tile([128, 512], F32, tag="acc", name=f"ps{i}")
          for i in range(8)]

    feats_t = feats.rearrange("(a p) d -> p a d", p=128)
    targets_t = targets.rearrange("(a p) c -> p a c", p=128)

    # ---- stream: bf16 feats/targets straight off HBM; scale targets ----
    tgs_raw = []
    for q in range(KT // 2):
        ft2 = ftp.tile([128, 2, D], BF16, tag="ft")
        nc.sync.dma_start(ft2[:], feats_t[:, 2 * q:2 * q + 2, :])
        tg2 = tgp.tile([128, 2, C], BF16, tag="tg", bufs=KT // 2,
                       name=f"tg{q}")
        nc.scalar.dma_start(tg2[:], targets_t[:, 2 * q:2 * q + 2, :])
        tgs_raw.append(tg2)
        for a in range(2):
            k = 2 * q + a
            ft = ft2[:, a, :]
            sq = sqp.tile([128, D], BF16, tag="sq")
            ssq = small.tile([128, 1], F32, tag="ssq")
            nc.scalar.activation(sq[:], ft, AF.Square, accum_out=ssq[:])
            nrm = small.tile([128, 1], F32, tag="nrm")
            nc.scalar.activation(nrm[:], ssq[:], AF.Sqrt, bias=eps2[:])
            rcp = small.tile([128, 1], F32, tag="rcp")
            nc.vector.reciprocal(rcp[:], nrm[:])
            tgs = tsp.tile([128, C], BF16, tag="tgs")
            nc.vector.tensor_scalar_mul(tgs[:], tg2[:, a, :], rcp[:])
            for c in range(4):
                lhs = tgs[:, c * 128:(c + 1) * 128]
                nc.tensor.matmul(ps[2 * c][:], lhs, ft[0:128, 0:512],
                                 start=(k == 0), stop=(k == KT - 1))
                nc.tensor.matmul(ps[2 * c + 1][:], lhs, ft[0:128, 512:1024],
                                 start=(k == 0), stop=(k == KT - 1))

    # ---- drain psum -> bf16 slabs; counts into ps[0]'s drained bank ----
    # slab c rows 0:64 = position block 2c, rows 64:128 = block 2c+1;
    # position block d belongs to core own^d. col 1024 carries counts.
    # All Tile-managed: the WAR of the counts matmuls on ps[0] (read by
    # slab0's drain) and the cross-iteration slab-buffer reuse (via the
    # critical's post_crit, which implies send completion) are tracked.
    # 12 bufs = 3-iteration rotation, so the critical's exit only needs
    # the PREVIOUS iteration's sends drained (slab used at it is reused
    # at it+3, gated by post_crit(it+1) which implies sends(it) done).
    slabs = [slabp.tile([128, SW], BF16, tag="slab", bufs=12,
                        name=f"slab{c}")
             for c in range(4)]
    for c in range(4):
        nc.vector.tensor_copy(slabs[c][:, 0:512], ps[2 * c][:])
        nc.scalar.copy(slabs[c][:, 512:1024], ps[2 * c + 1][:])
        if c == 0:
            # counts: free-size-1 matmuls re-using ps[0]'s bank
            # (start=True clears the region's has_written bits)
            for k in range(KT):
                tg = tgs_raw[k // 2][:, k % 2, :]
                for c2 in range(4):
                    nc.tensor.matmul(
                        ps[0][:, c2:c2 + 1],
                        tg[:, c2 * 128:(c2 + 1) * 128], ones[:],
                        start=(k == 0), stop=(k == KT - 1))
            cnt_sb = small.tile([128, 4], F32, tag="cnt_sb")
            nc.vector.tensor_copy(cnt_sb[:], ps[0][:, 0:4])
    for c in range(4):
        nc.scalar.copy(slabs[c][:, 1024:1025], cnt_sb[:, c:c + 1])

    t1 = redp.tile([128, SW], F32, tag="t1")
    t2 = redp.tile([128, SW], F32, tag="t2")
    accv = redp.tile([128, SW], F32, tag="accv")

    with tc.tile_critical():
        _emit_critical(nc, it, slabs, t1, t2, accv, recvs, sems)

    # ---- combine halves: shift odd half down via SBUF->SBUF DMA ----
    cmb = epi.tile([CL, SW], F32, tag="cmb")
    nc.sync.dma_start(cmb[:], accv[64:128, :])
    csf = epi.tile([CL, SW], F32, tag="csf")
    nc.vector.tensor_add(csf[:], accv[0:64, :], cmb[:])
    cs = csf[:, 0:1024]
    cnt = csf[:, 1024:1025]

    # ---- epilogue on this core's 64 classes ----
    mask = epi.tile([CL, 1], F32, tag="mask")
    nc.vector.tensor_scalar_min(mask[:], cnt, 1.0)
    omask = epi.tile([CL, 1], F32, tag="omask")
    nc.vector.tensor_scalar(omask[:], mask[:], -1.0, 1.0,
                            op0=ALU.mult, op1=ALU.add)
    inv = epi.tile([CL, 1], F32, tag="inv")
    nc.vector.tensor_scalar_max(inv[:], cnt, 1.0)
    nc.vector.reciprocal(inv[:], inv[:])
    new01 = epi.tile([CL, 1], F32, tag="new01")
    nc.vector.tensor_scalar(new01[:], ini[:], -1.0, 1.0,
                            op0=ALU.mult, op1=ALU.add)
    nc.vector.tensor_mul(new01[:], new01[:], mask[:])
    bco = epi.tile([CL, 1], F32, tag="bco")
    nc.vector.tensor_scalar(bco[:], new01[:], MOM, 1.0 - MOM,
                            op0=ALU.mult, op1=ALU.add)
    nc.vector.tensor_mul(bco[:], bco[:], mask[:])
    aco = epi.tile([CL, 1], F32, tag="aco")
    nc.vector.tensor_scalar(aco[:], bco[:], -1.0, 1.0,
                            op0=ALU.mult, op1=ALU.add)
    nc.vector.tensor_mul(bco[:], bco[:], inv[:])

    upd = epi.tile([CL, D], F32, tag="upd")
    nc.vector.tensor_scalar_mul(upd[:], ctr[:], aco[:])
    nc.vector.scalar_tensor_tensor(upd[:, 0:512], cs[:, 0:512], bco[:],
                                   upd[:, 0:512], op0=ALU.mult, op1=ALU.add)
    nc.vector.scalar_tensor_tensor(upd[:, 512:1024], cs[:, 512:1024], bco[:],
                                   upd[:, 512:1024],
                                   op0=ALU.mult, op1=ALU.add)

    usq = epi.tile([CL, D], F32, tag="usq")
    ussq_a = epi.tile([CL, 1], F32, tag="ussq_a")
    nc.scalar.activation(usq[:, 0:512], upd[:, 0:512], AF.Square,
                         accum_out=ussq_a[:])
    ussq_b = epi.tile([CL, 1], F32, tag="ussq_b")
    nc.scalar.activation(usq[:, 512:1024], upd[:, 512:1024], AF.Square,
                         accum_out=ussq_b[:])
    ussq = epi.tile([CL, 1], F32, tag="ussq")
    nc.vector.tensor_add(ussq[:], ussq_a[:], ussq_b[:])
    unrm = epi.tile([CL, 1], F32, tag="unrm")
    nc.scalar.activation(unrm[:], ussq[:], AF.Sqrt, bias=eps2[0:CL, :])
    urcp = epi.tile([CL, 1], F32, tag="urcp")
    nc.vector.reciprocal(urcp[:], unrm[:])
    nc.vector.tensor_scalar(urcp[:], mask[:], urcp[:], omask[:],
                            op0=ALU.mult, op1=ALU.add)

    ov = epi.tile([CL, D], F32, tag="ov")
    nc.vector.tensor_scalar_mul(ov[:, 0:512], upd[:, 0:512], urcp[:])
    nc.scalar.activation(ov[:, 512:1024], upd[:, 512:1024], AF.Copy,
                         scale=urcp[:])
    nc.sync.dma_start(out[:], ov[:])


def _emit_critical(nc, it, slabs, t1, t2, accv, recvs, sems):
    """Remote exchange, 8-way reduction and credit.

    gpsimd runs the sends/odd-half adds/credit; DVE runs the even-half
    adds. FIFO per engine; cross-core and cross-engine ordering by
    explicit semaphores with cumulative targets. recv tiles alternate by
    iteration parity, so sends of iteration it only require peers to
    have consumed iteration it-2. The final `sent` wait makes the
    critical's exit imply this iteration's sends were fully read off the
    slabs (slab pool bufs=8 rotates with period 2, so that is enough).
    """
    arr = sems["arr"]
    rv = recvs[it % 2]

    # ---- exchange: slab c -> relative dests 2c, 2c+1 (SBUF->SBUF) ----
    for d in range(1, NCORES):
        # ucode's D2D lane map applies a ^2 RMTV-balance to the relative
        # tpb for slots with bit 2 set — pre-compensate so the data lands
        # on the intended XOR destination (verified on HW).
        tpb = (d ^ 2) if d >= 4 else d
        rdests = [(0, tpb) if k == d else None for k in range(NCORES)]
        pr = nc.gpsimd.remote_dma_broadcast(
            rv[d][:, :], slabs[d // 2][:, :],
            remote_sem=arr[d], local_sem=sems["sent"], rdests=rdests)
        pr.then_inc(sems["prep"])
    nc.gpsimd.wait_ge(sems["prep"], 8 * it + 7)
    if it >= 2:
        # parity slot it%2 was last written at it-2: every peer must have
        # consumed it-2 (+2 credit from each of 7 per iteration).
        nc.gpsimd.wait_ge(sems["credit"], 14 * (it - 1))
    nc.gpsimd.trigger_dma(count=7)

    # ---- reduce: evens land rows 0:64 (DVE), odds rows 64:128 (Pool) --
    # (position block d sits in the sender's slab half d%2; my own block
    # is slab0 rows 0:64.)
    nc.vector.wait_ge(arr[2], 2 * (it + 1))
    nc.vector.tensor_add(t1[0:64, :], slabs[0][0:64, :], rv[2][0:64, :])
    nc.vector.wait_ge(arr[4], 2 * (it + 1))
    nc.vector.wait_ge(arr[6], 2 * (it + 1))
    v2 = nc.vector.tensor_add(t2[0:64, :], rv[4][0:64, :], rv[6][0:64, :])
    v2.then_inc(sems["evn"])    # all DVE-side recv reads done (FIFO)

    nc.gpsimd.wait_ge(arr[1], 2 * (it + 1))
    nc.gpsimd.wait_ge(arr[3], 2 * (it + 1))
    nc.gpsimd.tensor_add(t1[64:128, :], rv[1][64:128, :], rv[3][64:128, :])
    nc.gpsimd.wait_ge(arr[5], 2 * (it + 1))
    nc.gpsimd.wait_ge(arr[7], 2 * (it + 1))
    nc.gpsimd.tensor_add(t2[64:128, :], rv[5][64:128, :], rv[7][64:128, :])
    nc.gpsimd.wait_ge(sems["evn"], it + 1)
    nc.gpsimd.tensor_add(accv[:], t1[:], t2[:])

    # ---- credit the senders once this parity's recvs are re-usable ----
    crd = nc.gpsimd.remote_sem_update_broadcast(
        sems["credit"], sems["sent2"],
        rdests=[None] + [(0, (k ^ 2) if k >= 4 else k)
                         for k in range(1, NCORES)])
    crd.then_inc(sems["prep"])
    nc.gpsimd.wait_ge(sems["prep"], 8 * (it + 1))
    nc.gpsimd.trigger_dma(count=1)

    # previous iteration's sends fully read before exit (enough for the
    # 3-iteration slab rotation; takes this iteration's send-drain
    # latency off the critical exit)
    if it >= 1:
        nc.gpsimd.wait_ge(sems["sent"], 112 * it)


_NC_CACHE = None


def _get_nc():
    global _NC_CACHE
    if _NC_CACHE is None:
        _NC_CACHE = build_nc()
    return _NC_CACHE


_PERMS = None


def _perms():
    global _PERMS
    if _PERMS is None:
        _PERMS = []
        for r in range(NCORES):
            p = np.concatenate([np.arange(CL) + CL * (r ^ d)
                                for d in range(NCORES)])
            _PERMS.append(p)
    return _PERMS


def run_spmd(feats, targets, centers, initialized, **kw):
    feats = np.asarray(feats, dtype=np.float32)
    targets = np.asarray(targets, dtype=np.float32)
    centers = np.ascontiguousarray(np.asarray(centers, dtype=np.float32))
    init_f = np.asarray(initialized).astype(np.float32).reshape(C, 1)
    assert feats.shape == (B, D) and targets.shape == (B, C)
    assert centers.shape == (C, D)

    bf16 = ml_dtypes.bfloat16
    perms = _perms()
    nc = _get_nc()
    in_maps = []
    for r in range(NCORES):
        in_maps.append({
            "feats": np.ascontiguousarray(
                feats[r * BL:(r + 1) * BL].astype(bf16)),
            "targets": np.ascontiguousarray(
                targets[r * BL:(r + 1) * BL][:, perms[r]].astype(bf16)),
            "centers": np.ascontiguousarray(centers[r * CL:(r + 1) * CL]),
            "inited": np.ascontiguousarray(init_f[r * CL:(r + 1) * CL]),
        })
    res = run_bass_kernel_spmd(nc, in_maps, core_ids=list(range(NCORES)), **kw)
    out = np.concatenate([res.results[r]["out"] for r in range(NCORES)], axis=0)
    return out.astype(np.float32), res


def kernel(feats, targets, centers, initialized):
    out, _ = run_spmd(feats, targets, centers, initialized)
    return out


# revision 4
# speedup vs baseline: 1.1936x; 1.1936x over previous
"""Trainium2 Bass kernel for ClassCentersEMA (vq_codebook) — v3.

Reference semantics (B=16384, D=1024, C=512):
    feats_n   = feats / max(||feats||_row, eps)
    counts    = targets.sum(0)                       # [C]
    class_sums= targets^T @ feats_n                  # [C, D]
    mask      = counts > 0
    means     = class_sums / max(counts, 1)          # rows 0 where !mask
    new       = !initialized & mask
    base      = where(new, means, centers)
    blended   = 0.9*base + 0.1*means
    upd       = where(mask, blended, centers)
    out       = where(mask, upd / max(||upd||, eps), upd)

v3 vs the v2 DRAM-AllToAll design (59us):
  - Host pre-casts feats and targets to bf16: the HBM input stream drops
    from 12.3 MiB to 6.25 MiB per core; the PE bf16 matmul (~28us)
    becomes the critical path instead of DMA.
  - The cross-core reduction of the [C, D] partials goes SBUF->SBUF via
    remote_dma_broadcast (SWDGE) instead of bouncing 8.3 MiB per core
    through HBM + an ncfw AllToAll. targets columns are permuted on the
    HOST per core (position block d holds the classes of core r^d), so
    every core runs the identical program and relative XOR destinations
    line up with class ownership: slab c (psum pair c) rows 0:64 belong
    to relative dest 2c, rows 64:128 to dest 2c+1; core r^d receives my
    slab at recv[d] and uses the half selected by d's parity.
  - Counts ride as one extra bf16 column (exact: per-core partial counts
    are small integers << 256) on each 128x1028 slab.
  - The exchange, counts matmuls, psum drains and the 8-way reduction
    live in a tc.tile_critical() section with manual semaphores (Tile's
    build-time scheduling sim cannot model cross-core semaphore
    increments; inside a critical section engines run FIFO and the sim
    does not trace the waits).
  - A tiny AllGather at NEFF start is a cross-core rendezvous: it
    guarantees every core has started (and had its semaphores reset)
    before any remote increment can land. gpsimd-only, overlapped with
    the input stream, once per dispatch.
"""

import numpy as np
import ml_dtypes

import concourse.bass as bass
import concourse.mybir as mybir
import concourse.tile as tile
from concourse import bacc
from concourse.bass_utils import run_bass_kernel_spmd

F32 = mybir.dt.float32
BF16 = mybir.dt.bfloat16
AF = mybir.ActivationFunctionType
ALU = mybir.AluOpType

NCORES = 8
B, D, C = 16384, 1024, 512
BL = B // NCORES          # 2048 rows per core
KT = BL // 128            # 16 k-tiles of 128
CL = C // NCORES          # 64 classes per core
MOM = 0.9
EPS = 1e-12
SW = 1028                 # slab width: 1024 D + 1 counts + 3 pad


def build_nc(niters=1, use_coll=True):
    """niters>1 unrolls the kernel body N times in one NEFF — used only
    for timing (slope over N isolates device exec time)."""
    nc = bacc.Bacc("TRN2", target_bir_lowering=False, debug=False,
                   num_devices=NCORES)

    feats = nc.dram_tensor("feats", [BL, D], BF16, kind="ExternalInput")
    targets = nc.dram_tensor("targets", [BL, C], BF16, kind="ExternalInput")
    centers = nc.dram_tensor("centers", [CL, D], F32, kind="ExternalInput")
    inited = nc.dram_tensor("inited", [CL, 1], F32, kind="ExternalInput")
    out = nc.dram_tensor("out", [CL, D], F32, kind="ExternalOutput")

    rg = [list(range(NCORES))]

    sems = dict(
        arr=[nc.alloc_semaphore(f"arr{d}") for d in range(NCORES)],
        sent=nc.alloc_semaphore("sent"),
        sent2=nc.alloc_semaphore("sent2"),
        credit=nc.alloc_semaphore("credit"),
        prep=nc.alloc_semaphore("prep"),
        evn=nc.alloc_semaphore("evn"),
    )

    with tile.TileContext(nc) as tc:
        with (
            tc.tile_pool(name="dram", bufs=1, space="DRAM") as dram,
            tc.tile_pool(name="ftp", bufs=3) as ftp,
            tc.tile_pool(name="tgp", bufs=1) as tgp,
            tc.tile_pool(name="tsp", bufs=3) as tsp,
            tc.tile_pool(name="sq", bufs=2) as sqp,
            tc.tile_pool(name="small", bufs=6) as small,
            tc.tile_pool(name="single", bufs=1) as single,
            tc.tile_pool(name="slab", bufs=2) as slabp,
            tc.tile_pool(name="recv", bufs=1) as recvp,
            tc.tile_pool(name="red", bufs=1) as redp,
            tc.tile_pool(name="psum", bufs=8, space="PSUM") as ppool,
            tc.tile_pool(name="epi", bufs=1) as epi,
        ):
            io = dict(feats=feats, targets=targets, centers=centers,
                      inited=inited, out=out)

            ones = single.tile([128, 1], BF16, name="ones")
            nc.vector.memset(ones[:], 1.0)
            eps2 = single.tile([128, 1], F32, name="eps2")
            nc.vector.memset(eps2[:], EPS * EPS)
            consts = dict(ones=ones, eps2=eps2)

            # entry rendezvous (see module docstring)
            bar_in = dram.tile([1, 64], F32, name="bar_in")
            bar_out = dram.tile([NCORES, 64], F32, name="bar_out")
            nc.gpsimd.collective_compute(
                "AllGather", ALU.bypass, replica_groups=rg,
                ins=[bar_in[:].opt()], outs=[bar_out[:].opt()])

            # persistent recv tiles — remote cores write here; addresses
            # are identical on every core (identical program). Two parity
            # sets: iteration it uses set it%2, so sends need peers'
            # consumption of it-2 (not it-1) — the credit roundtrip
            # overlaps a full iteration.
            recvs = [[None] + [recvp.tile([128, SW], BF16,
                                          name=f"recv{par}_{d}")
                               for d in range(1, NCORES)]
                     for par in range(2)]

            pools = dict(ftp=ftp, tgp=tgp, tsp=tsp, sqp=sqp, small=small,
                         slabp=slabp, redp=redp, ppool=ppool, epi=epi)
            for i in range(niters):
                _emit_iteration(nc, tc, i, io, consts, pools, recvs, sems)

    nc.compile()
    return nc


def _emit_iteration(nc, tc, it, io, consts, pools, recvs, sems):
    feats, targets = io["feats"], io["targets"]
    centers, inited, out = io["centers"], io["inited"], io["out"]
    ones, eps2 = consts["ones"], consts["eps2"]
    ftp, tgp, tsp = pools["ftp"], pools["tgp"], pools["tsp"]
    sqp, small = pools["sqp"], pools["small"]
    slabp, redp, ppool, epi = (pools["slabp"], pools["redp"], pools["ppool"],
                               pools["epi"])
    arr = sems["arr"]

    # epilogue inputs that depend on nothing — issue DMAs up front
    ctr = epi.tile([CL, D], F32, tag="ctr")
    nc.scalar.dma_start(ctr[:], centers[:])
    ini = epi.tile([CL, 1], F32, tag="ini")
    nc.scalar.dma_start(ini[:], inited[:])

    # 8 PSUM banks: (position-block pair, D-half) over all 16 k-tiles
    ps = [ppool.tile([128, 512], F32, tag="acc", name=f"ps{i}")
          for i in range(8)]

    feats_t = feats.rearrange("(a p) d -> p a d", p=128)
    targets_t = targets.rearrange("(a p) c -> p a c", p=128)

    # ---- stream: bf16 feats/targets straight off HBM; scale targets ----
    tgs_raw = []
    for q in range(KT // 2):
        ft2 = ftp.tile([128, 2, D], BF16, tag="ft")
        nc.sync.dma_start(ft2[:], feats_t[:, 2 * q:2 * q + 2, :])
        tg2 = tgp.tile([128, 2, C], BF16, tag="tg", bufs=KT // 2,
                       name=f"tg{q}")
        nc.scalar.dma_start(tg2[:], targets_t[:, 2 * q:2 * q + 2, :])
        tgs_raw.append(tg2)
        for a in range(2):
            k = 2 * q + a
            ft = ft2[:, a, :]
            sq = sqp.tile([128, D], BF16, tag="sq")
            ssq = small.tile([128, 1], F32, tag="ssq")
            nc.scalar.activation(sq[:], ft, AF.Square, accum_out=ssq[:])
            nrm = small.tile([128, 1], F32, tag="nrm")
            nc.scalar.activation(nrm[:], ssq[:], AF.Sqrt, bias=eps2[:])
            rcp = small.tile([128, 1], F32, tag="rcp")
            nc.vector.reciprocal(rcp[:], nrm[:])
            tgs = tsp.tile([128, C], BF16, tag="tgs")
            nc.vector.tensor_scalar_mul(tgs[:], tg2[:, a, :], rcp[:])
            for c in range(4):
                lhs = tgs[:, c * 128:(c + 1) * 128]
                nc.tensor.matmul(ps[2 * c][:], lhs, ft[0:128, 0:512],
                                 start=(k == 0), stop=(k == KT - 1))
                nc.tensor.matmul(ps[2 * c + 1][:], lhs, ft[0:128, 512:1024],
                                 start=(k == 0), stop=(k == KT - 1))

    # ---- drain psum -> bf16 slabs; counts into ps[0]'s drained bank ----
    # slab c rows 0:64 = position block 2c, rows 64:128 = block 2c+1;
    # position block d belongs to core own^d. col 1024 carries counts.
    # All Tile-managed: the WAR of the counts matmuls on ps[0] (read by
    # slab0's drain) and the cross-iteration slab-buffer reuse (via the
    # critical's post_crit, which implies send completion) are tracked.
    slabs = [slabp.tile([128, SW], BF16, tag="slab", bufs=8,
                        name=f"slab{c}")
             for c in range(4)]
    for c in range(4):
        nc.vector.tensor_copy(slabs[c][:, 0:512], ps[2 * c][:])
        nc.scalar.copy(slabs[c][:, 512:1024], ps[2 * c + 1][:])
        if c == 0:
            # counts: free-size-1 matmuls re-using ps[0]'s bank
            # (start=True clears the region's has_written bits)
            for k in range(KT):
                tg = tgs_raw[k // 2][:, k % 2, :]
                for c2 in range(4):
                    nc.tensor.matmul(
                        ps[0][:, c2:c2 + 1],
                        tg[:, c2 * 128:(c2 + 1) * 128], ones[:],
                        start=(k == 0), stop=(k == KT - 1))
            cnt_sb = small.tile([128, 4], F32, tag="cnt_sb")
            nc.vector.tensor_copy(cnt_sb[:], ps[0][:, 0:4])
    for c in range(4):
        nc.scalar.copy(slabs[c][:, 1024:1025], cnt_sb[:, c:c + 1])

    t1 = redp.tile([128, SW], F32, tag="t1")
    t2 = redp.tile([128, SW], F32, tag="t2")
    accv = redp.tile([128, SW], F32, tag="accv")

    with tc.tile_critical():
        _emit_critical(nc, it, slabs, t1, t2, accv, recvs, sems)

    # ---- combine halves: shift odd half down via SBUF->SBUF DMA ----
    cmb = epi.tile([CL, SW], F32, tag="cmb")
    nc.sync.dma_start(cmb[:], accv[64:128, :])
    csf = epi.tile([CL, SW], F32, tag="csf")
    nc.vector.tensor_add(csf[:], accv[0:64, :], cmb[:])
    cs = csf[:, 0:1024]
    cnt = csf[:, 1024:1025]

    # ---- epilogue on this core's 64 classes ----
    mask = epi.tile([CL, 1], F32, tag="mask")
    nc.vector.tensor_scalar_min(mask[:], cnt, 1.0)
    omask = epi.tile([CL, 1], F32, tag="omask")
    nc.vector.tensor_scalar(omask[:], mask[:], -1.0, 1.0,
                            op0=ALU.mult, op1=ALU.add)
    inv = epi.tile([CL, 1], F32, tag="inv")
    nc.vector.tensor_scalar_max(inv[:], cnt, 1.0)
    nc.vector.reciprocal(inv[:], inv[:])
    new01 = epi.tile([CL, 1], F32, tag="new01")
    nc.vector.tensor_scalar(new01[:], ini[:], -1.0, 1.0,
                            op0=ALU.mult, op1=ALU.add)
    nc.vector.tensor_mul(new01[:], new01[:], mask[:])
    bco = epi.tile([CL, 1], F32, tag="bco")
    nc.vector.tensor_scalar(bco[:], new01[:], MOM, 1.0 - MOM,
                            op0=ALU.mult, op1=ALU.add)
    nc.vector.tensor_mul(bco[:], bco[:], mask[:])
    aco = epi.tile([CL, 1], F32, tag="aco")
    nc.vector.tensor_scalar(aco[:], bco[:], -1.0, 1.0,
                            op0=ALU.mult, op1=ALU.add)
    nc.vector.tensor_mul(bco[:], bco[:], inv[:])

    upd = epi.tile([CL, D], F32, tag="upd")
    nc.vector.tensor_scalar_mul(upd[:], ctr[:], aco[:])
    nc.vector.scalar_tensor_tensor(upd[:, 0:512], cs[:, 0:512], bco[:],
                                   upd[:, 0:512], op0=ALU.mult, op1=ALU.add)
    nc.vector.scalar_tensor_tensor(upd[:, 512:1024], cs[:, 512:1024], bco[:],
                                   upd[:, 512:1024],
                                   op0=ALU.mult, op1=ALU.add)

    usq = epi.tile([CL, D], F32, tag="usq")
    ussq_a = epi.tile([CL, 1], F32, tag="ussq_a")
    nc.scalar.activation(usq[:, 0:512], upd[:, 0:512], AF.Square,
                         accum_out=ussq_a[:])
    ussq_b = epi.tile([CL, 1], F32, tag="ussq_b")
    nc.scalar.activation(usq[:, 512:1024], upd[:, 512:1024], AF.Square,
                         accum_out=ussq_b[:])
    ussq = epi.tile([CL, 1], F32, tag="ussq")
    nc.vector.tensor_add(ussq[:], ussq_a[:], ussq_b[:])
    unrm = epi.tile([CL, 1], F32, tag="unrm")
    nc.scalar.activation(unrm[:], ussq[:], AF.Sqrt, bias=eps2[0:CL, :])
    urcp = epi.tile([CL, 1], F32, tag="urcp")
    nc.vector.reciprocal(urcp[:], unrm[:])
    nc.vector.tensor_scalar(urcp[:], mask[:], urcp[:], omask[:],
                            op0=ALU.mult, op1=ALU.add)

    ov = epi.tile([CL, D], F32, tag="ov")
    nc.vector.tensor_scalar_mul(ov[:, 0:512], upd[:, 0:512], urcp[:])
    nc.scalar.activation(ov[:, 512:1024], upd[:, 512:1024], AF.Copy,
                         scale=urcp[:])
    nc.sync.dma_start(out[:], ov[:])


def _emit_critical(nc, it, slabs, t1, t2, accv, recvs, sems):
    """Remote exchange, 8-way reduction and credit.

    gpsimd runs the sends/odd-half adds/credit; DVE runs the even-half
    adds. FIFO per engine; cross-core and cross-engine ordering by
    explicit semaphores with cumulative targets. recv tiles alternate by
    iteration parity, so sends of iteration it only require peers to
    have consumed iteration it-2. The final `sent` wait makes the
    critical's exit imply this iteration's sends were fully read off the
    slabs (slab pool bufs=8 rotates with period 2, so that is enough).
    """
    arr = sems["arr"]
    rv = recvs[it % 2]

    # ---- exchange: slab c -> relative dests 2c, 2c+1 (SBUF->SBUF) ----
    for d in range(1, NCORES):
        # ucode's D2D lane map applies a ^2 RMTV-balance to the relative
        # tpb for slots with bit 2 set — pre-compensate so the data lands
        # on the intended XOR destination (verified on HW).
        tpb = (d ^ 2) if d >= 4 else d
        rdests = [(0, tpb) if k == d else None for k in range(NCORES)]
        pr = nc.gpsimd.remote_dma_broadcast(
            rv[d][:, :], slabs[d // 2][:, :],
            remote_sem=arr[d], local_sem=sems["sent"], rdests=rdests)
        pr.then_inc(sems["prep"])
    nc.gpsimd.wait_ge(sems["prep"], 8 * it + 7)
    if it >= 2:
        # parity slot it%2 was last written at it-2: every peer must have
        # consumed it-2 (+2 credit from each of 7 per iteration).
        nc.gpsimd.wait_ge(sems["credit"], 14 * (it - 1))
    nc.gpsimd.trigger_dma(count=7)

    # ---- reduce: evens land rows 0:64 (DVE), odds rows 64:128 (Pool) --
    # (position block d sits in the sender's slab half d%2; my own block
    # is slab0 rows 0:64.)
    nc.vector.wait_ge(arr[2], 2 * (it + 1))
    nc.vector.tensor_add(t1[0:64, :], slabs[0][0:64, :], rv[2][0:64, :])
    nc.vector.wait_ge(arr[4], 2 * (it + 1))
    nc.vector.wait_ge(arr[6], 2 * (it + 1))
    v2 = nc.vector.tensor_add(t2[0:64, :], rv[4][0:64, :], rv[6][0:64, :])
    v2.then_inc(sems["evn"])    # all DVE-side recv reads done (FIFO)

    nc.gpsimd.wait_ge(arr[1], 2 * (it + 1))
    nc.gpsimd.wait_ge(arr[3], 2 * (it + 1))
    nc.gpsimd.tensor_add(t1[64:128, :], rv[1][64:128, :], rv[3][64:128, :])
    nc.gpsimd.wait_ge(arr[5], 2 * (it + 1))
    nc.gpsimd.wait_ge(arr[7], 2 * (it + 1))
    nc.gpsimd.tensor_add(t2[64:128, :], rv[5][64:128, :], rv[7][64:128, :])
    nc.gpsimd.wait_ge(sems["evn"], it + 1)
    nc.gpsimd.tensor_add(accv[:], t1[:], t2[:])

    # ---- credit the senders once this parity's recvs are re-usable ----
    crd = nc.gpsimd.remote_sem_update_broadcast(
        sems["credit"], sems["sent2"],
        rdests=[None] + [(0, (k ^ 2) if k >= 4 else k)
                         for k in range(1, NCORES)])
    crd.then_inc(sems["prep"])
    nc.gpsimd.wait_ge(sems["prep"], 8 * (it + 1))
    nc.gpsimd.trigger_dma(count=1)

    # this iteration's sends fully read (7 x +16 on `sent`) before exit
    nc.gpsimd.wait_ge(sems["sent"], 112 * (it + 1))


_NC_CACHE = None


def _get_nc():
    global _NC_CACHE
    if _NC_CACHE is None:
        _NC_CACHE = build_nc()
    return _NC_CACHE


_PERMS = None


def _perms():
    global _PERMS
    if _PERMS is None:
        _PERMS = []
        for r in range(NCORES):
            p = np.concatenate([np.arange(CL) + CL * (r ^ d)
                                for d in range(NCORES)])
            _PERMS.append(p)
    return _PERMS


def run_spmd(feats, targets, centers, initialized, **kw):
    feats = np.asarray(feats, dtype=np.float32)
    targets = np.asarray(targets, dtype=np.float32)
    centers = np.ascontiguousarray(np.asarray(centers, dtype=np.float32))
    init_f = np.asarray(initialized).astype(np.float32).reshape(C, 1)
    assert feats.shape == (B, D) and targets.shape == (B, C)
    assert centers.shape == (C, D)

    bf16 = ml_dtypes.bfloat16
    perms = _perms()
    nc = _get_nc()
    in_maps = []
    for r in range(NCORES):
        in_maps.append({
            "feats": np.ascontiguousarray(
                feats[r * BL:(r + 1) * BL].astype(bf16)),
            "targets": np.ascontiguousarray(
                targets[r * BL:(r + 1) * BL][:, perms[r]].astype(bf16)),
            "centers": np.ascontiguousarray(centers[r * CL:(r + 1) * CL]),
            "inited": np.ascontiguousarray(init_f[r * CL:(r + 1) * CL]),
        })
    res = run_bass_kernel_spmd(nc, in_maps, core_ids=list(range(NCORES)), **kw)
    out = np.concatenate([res.results[r]["out"] for r in range(NCORES)], axis=0)
    return out.astype(np.float32), res


def kernel(feats, targets, centers, initialized):
    out, _ = run_spmd(feats, targets, centers, initialized)
    return out
